# revision 2
# baseline (speedup 1.0000x reference)
"""Trainium2 Bass kernel for nn_CortexNetwork (dense_cnn, memory-bound).

Reference computation:
    patches[c,i,j,u,v] = x[c, rx[i]+u, ry[j]+v]
    aff[i,j] = sum_{c,u,v} patches * Wa
    exc[i,j] = sum_c prev[c,i,j] * sum_{x,y} We[c,i,j,x,y]   (inh likewise, Wi)
    out      = broadcast_c(relu(aff + 0.9*exc - 0.9*inh))

Strategy: tensor-parallel over the 36x36=1296 grid units, 162 units per
core on 8 cores; every reduction is unit-local so there are no
collectives.  The host lays each core's data out as 21 tiles of
[128 partitions = 16 channels x 8 units,
 3744 columns  = We(1296) | Wi(1296) | Wa(576) | patch(576)]
so the device sees one linear ~1.9MB DMA per tile.  On device the
free-dim reductions are split across ScalarE (activation with
scale=+-0.9*prev and accum_out, 15 tiles) and VectorE (tensor_reduce +
per-partition multiply, 6 tiles; plus all afferent mul+reduce); the
final sum over the 16 channel partitions is a 0/1-selector matmul on
the tensor engine, then relu.
"""

import numpy as np

import concourse.bass as bass
import concourse.bacc as bacc
import concourse.mybir as mybir
from concourse import tile
from concourse.bass_utils import run_bass_kernel_spmd

N_CORES = 8
C = 16
GX = GY = 36
RF = 24
IMG = 64
GAMMA = 0.9

UNITS = GX * GY                  # 1296
PER_CORE = UNITS // N_CORES      # 162
S = 8                            # units per tile (partition dim = C*S = 128)
T = (PER_CORE + S - 1) // S      # 21 tiles, last holds 2 real units
PAD_UNITS = T * S                # 168
ACT_T = 15                       # tiles whose lateral reduction runs on ScalarE
DVE_T = T - ACT_T                # 6 tiles on VectorE
FW = GX * GY                     # lateral free size per channel: 1296
FA = RF * RF                     # afferent free size per channel: 576
COLS = 2 * FW + 2 * FA           # 3744

_PROGRAM_CACHE = {}


def _build_program():
    f32 = mybir.dt.float32
    AL = mybir.AluOpType
    AF = mybir.ActivationFunctionType
    AX = mybir.AxisListType

    nc = bacc.Bacc(
        "TRN2", target_bir_lowering=False, debug=False, num_devices=N_CORES
    )
    big = nc.dram_tensor("big", [T, 128, COLS], f32, kind="ExternalInput").ap()
    possb_d = nc.dram_tensor("possb", [128, T], f32, kind="ExternalInput").ap()
    negsb_d = nc.dram_tensor("negsb", [128, T], f32, kind="ExternalInput").ap()
    sel_d = nc.dram_tensor("sel", [128, S], f32, kind="ExternalInput").ap()
    out_d = nc.dram_tensor("out", [S, T], f32, kind="ExternalOutput").ap()

    with tile.TileContext(nc) as tc:
        with (
            tc.tile_pool(name="w", bufs=6) as wp,
            tc.tile_pool(name="cst", bufs=1) as cp,
            tc.tile_pool(name="junk", bufs=3) as jp,
            tc.tile_pool(name="acc", bufs=3) as accp,
            tc.tile_pool(name="fin", bufs=1) as fp,
            tc.tile_pool(name="ps", bufs=1, space="PSUM") as pp,
        ):
            possb = cp.tile([128, T], f32, tag="possb")
            negsb = cp.tile([128, T], f32, tag="negsb")
            sel = cp.tile([128, S], f32, tag="sel")
            pact = cp.tile([128, 2 * ACT_T], f32, tag="pact")
            pvec = cp.tile([128, T + 2 * DVE_T], f32, tag="pvec")
            nc.sync.dma_start(possb[:], possb_d[:])
            nc.sync.dma_start(negsb[:], negsb_d[:])
            nc.sync.dma_start(sel[:], sel_d[:])

            for t in range(T):
                w = wp.tile([128, COLS], f32, tag="w")
                nc.sync.dma_start(w[:], big[t])
                we = w[:, 0:FW]
                wi = w[:, FW:2 * FW]
                wa = w[:, 2 * FW:2 * FW + FA]
                pt = w[:, 2 * FW + FA:COLS]
                if t < ACT_T:
                    # ScalarE: accum_out = sum_free(we * 0.9*prev) per partition
                    ja = jp.tile([128, FW], f32, tag="jlat1")
                    nc.scalar.activation(
                        ja[:], we, AF.Copy,
                        scale=possb[:, t:t + 1],
                        accum_out=pact[:, 2 * t:2 * t + 1],
                    )
                    jb = jp.tile([128, FW], f32, tag="jlat2")
                    nc.scalar.activation(
                        jb[:], wi, AF.Copy,
                        scale=negsb[:, t:t + 1],
                        accum_out=pact[:, 2 * t + 1:2 * t + 2],
                    )
                else:
                    # VectorE: unweighted free-dim reduce, then a tiny
                    # per-partition multiply by +-0.9*prev.
                    d = t - ACT_T
                    r1 = accp.tile([128, 1], f32, tag="r1")
                    nc.vector.tensor_reduce(r1[:], we, axis=AX.X, op=AL.add)
                    nc.vector.tensor_mul(
                        pvec[:, T + 2 * d:T + 2 * d + 1], r1[:],
                        possb[:, t:t + 1],
                    )
                    r2 = accp.tile([128, 1], f32, tag="r2")
                    nc.vector.tensor_reduce(r2[:], wi, axis=AX.X, op=AL.add)
                    nc.vector.tensor_mul(
                        pvec[:, T + 2 * d + 1:T + 2 * d + 2], r2[:],
                        negsb[:, t:t + 1],
                    )
                # afferent: elementwise product then free-dim reduce (VectorE)
                prod = jp.tile([128, FA], f32, tag="prod")
                nc.vector.tensor_mul(prod[:], wa, pt)
                nc.vector.tensor_reduce(
                    pvec[:, t:t + 1], prod[:], axis=AX.X, op=AL.add
                )

            # Sum over the 16 channel partitions: psum[s, col] =
            # sum_p sel[p, s] * partials[p, col] with sel[p,s] = (p%8==s).
            NPA = 2 * ACT_T                  # 30
            NPV = T + 2 * DVE_T              # 33
            psum = pp.tile([S, NPA + NPV], f32, tag="ps")
            nc.tensor.matmul(psum[:, 0:NPA], sel[:], pact[:],
                             start=True, stop=True)
            nc.tensor.matmul(psum[:, NPA:NPA + NPV], sel[:], pvec[:],
                             start=True, stop=True)

            res = fp.tile([S, T], f32, tag="res")
            tmp1 = fp.tile([S, ACT_T], f32, tag="tmp1")
            tmp2 = fp.tile([S, DVE_T], f32, tag="tmp2")
            # ACT tiles: lateral pair sums live in psum[:, 0:30]
            nc.vector.tensor_reduce(
                tmp1[:],
                psum[:, 0:NPA].rearrange("p (a b) -> p a b", b=2),
                axis=AX.X, op=AL.add,
            )
            nc.vector.tensor_add(
                res[:, 0:ACT_T], tmp1[:], psum[:, NPA:NPA + ACT_T]
            )
            # DVE tiles: lateral pair sums live in psum[:, 30+21 : 30+33]
            nc.vector.tensor_reduce(
                tmp2[:],
                psum[:, NPA + T:NPA + NPV].rearrange("p (a b) -> p a b", b=2),
                axis=AX.X, op=AL.add,
            )
            nc.vector.tensor_add(
                res[:, ACT_T:T], tmp2[:], psum[:, NPA + ACT_T:NPA + T]
            )
            nc.vector.tensor_scalar_max(res[:], res[:], 0.0)
            nc.sync.dma_start(out_d[:], res[:])

    nc.compile()
    return nc


def _get_program():
    if "nc" not in _PROGRAM_CACHE:
        _PROGRAM_CACHE["nc"] = _build_program()
    return _PROGRAM_CACHE["nc"]


def _prep_in_maps(inputs):
    x = np.asarray(inputs["x"], dtype=np.float32)
    prev = np.asarray(inputs["prev_activity"], dtype=np.float32)
    wa = np.asarray(inputs["afferent_weights"], dtype=np.float32).reshape(C, UNITS, FA)
    we = np.asarray(inputs["ex_lateral_weights"], dtype=np.float32).reshape(C, UNITS, FW)
    wi = np.asarray(inputs["in_lateral_weights"], dtype=np.float32).reshape(C, UNITS, FW)
    rx = np.asarray(inputs["rx"]).astype(np.int64)
    ry = np.asarray(inputs["ry"]).astype(np.int64)

    u = np.arange(RF)
    ix = rx[:, None] + u                     # [GX, RF]
    iy = ry[:, None] + u                     # [GY, RF]
    px = x[:, ix, :]                         # [C, GX, RF, IMG]
    patches = px[:, :, :, iy]                # [C, GX, RF, GY, RF]
    patches = np.ascontiguousarray(patches.transpose(0, 1, 3, 2, 4))
    patches = patches.reshape(C, UNITS, FA)
    prevf = prev.reshape(C, UNITS)

    sel = (np.arange(128)[:, None] % S == np.arange(S)[None, :]).astype(np.float32)

    def tilefy(a, F, n0):
        # [C, UNITS, F] -> [T, C*S, F] with partition p = c*S + s
        s = a[:, n0:n0 + PER_CORE]
        pad = np.zeros((C, PAD_UNITS - PER_CORE, F), np.float32)
        s = np.concatenate([s, pad], axis=1)
        return s.reshape(C, T, S, F).transpose(1, 0, 2, 3).reshape(T, C * S, F)

    in_maps = []
    for k in range(N_CORES):
        n0 = k * PER_CORE
        big = np.concatenate(
            [tilefy(we, FW, n0), tilefy(wi, FW, n0),
             tilefy(wa, FA, n0), tilefy(patches, FA, n0)],
            axis=2,
        )
        pv = prevf[:, n0:n0 + PER_CORE]
        pv = np.concatenate(
            [pv, np.zeros((C, PAD_UNITS - PER_CORE), np.float32)], axis=1
        )
        pv = pv.reshape(C, T, S).transpose(0, 2, 1).reshape(C * S, T)
        in_maps.append({
            "big": np.ascontiguousarray(big),
            "possb": np.ascontiguousarray(GAMMA * pv),
            "negsb": np.ascontiguousarray(-GAMMA * pv),
            "sel": sel,
        })
    return in_maps


def _assemble_output(results):
    act = np.empty(UNITS, np.float32)
    for k in range(N_CORES):
        o = np.asarray(results[k]["out"])            # [S, T]
        loc = o.T.reshape(PAD_UNITS)[:PER_CORE]      # unit n_local = 8t + s
        act[k * PER_CORE:(k + 1) * PER_CORE] = loc
    out = np.broadcast_to(act.reshape(1, GX, GY), (C, GX, GY))
    return np.ascontiguousarray(out, dtype=np.float32)


def kernel(**inputs):
    nc = _get_program()
    in_maps = _prep_in_maps(inputs)
    res = run_bass_kernel_spmd(nc, in_maps, core_ids=list(range(N_CORES)))
    return _assemble_output(res.results)


# revision 6
# speedup vs baseline: 1.0237x; 1.0237x over previous
"""Trainium2 Bass kernel for nn_CortexNetwork (dense_cnn, memory-bound).

Reference computation:
    patches[c,i,j,u,v] = x[c, rx[i]+u, ry[j]+v]
    aff[i,j] = sum_{c,u,v} patches * Wa
    exc[i,j] = sum_c prev[c,i,j] * sum_{x,y} We[c,i,j,x,y]   (inh likewise, Wi)
    out      = broadcast_c(relu(aff + 0.9*exc - 0.9*inh))

Strategy: tensor-parallel over the 36x36=1296 grid units, 162 units per
core on 8 cores; every reduction is unit-local so there are no
collectives.  The host lays each core's data out as 20 tiles of
[128 partitions = 16 channels x 8 units,
 3744 columns  = We(1296) | -Wi(1296) | Wa(576) | patch(576)]
plus one 32-partition tile for the 2 leftover units, so the device sees
one linear ~1.9MB DMA per tile.  Wi is negated on the host so the whole
lateral term is one reduction: 0.9*prev * sum(We|-Wi).  The free-dim
reductions are split across ScalarE (activation with scale=0.9*prev and
accum_out) and VectorE (tensor_reduce + per-partition multiply), with
ownership interleaved over tiles so both engines drain with the DMA
stream; all afferent products run on VectorE.  The final sum over the
16 channel partitions is a 0/1-selector matmul on the tensor engine,
then relu.
"""

import numpy as np

import concourse.bass as bass
import concourse.bacc as bacc
import concourse.mybir as mybir
from concourse import tile
from concourse.bass_utils import run_bass_kernel_spmd

N_CORES = 8
C = 16
GX = GY = 36
RF = 24
IMG = 64
GAMMA = 0.9

UNITS = GX * GY                  # 1296
PER_CORE = UNITS // N_CORES      # 162
S = 8                            # units per full tile (partition dim C*S=128)
TF = PER_CORE // S               # 20 full tiles
S2 = PER_CORE - TF * S           # 2 units in the last (32-partition) tile
T = TF + 1                       # 21 tiles total
FW = GX * GY                     # lateral free size per channel: 1296
FA = RF * RF                     # afferent free size per channel: 576
COLS = 2 * FW + 2 * FA           # 3744
# Full tiles whose lateral reduction runs on VectorE, spread through the
# stream so ScalarE and VectorE drain together; the rest go to ScalarE.
DVE_TILES = (2, 6, 9, 13, 16, 19)

_PROGRAM_CACHE = {}


def _build_program():
    f32 = mybir.dt.float32
    AL = mybir.AluOpType
    AF = mybir.ActivationFunctionType
    AX = mybir.AxisListType

    nc = bacc.Bacc(
        "TRN2", target_bir_lowering=False, debug=False, num_devices=N_CORES
    )
    big = nc.dram_tensor("big", [TF, 128, COLS], f32, kind="ExternalInput").ap()
    big2_d = nc.dram_tensor("big2", [C * S2, COLS], f32, kind="ExternalInput").ap()
    possb_d = nc.dram_tensor("possb", [128, TF], f32, kind="ExternalInput").ap()
    possb2_d = nc.dram_tensor("possb2", [C * S2, 1], f32, kind="ExternalInput").ap()
    sel_d = nc.dram_tensor("sel", [128, S], f32, kind="ExternalInput").ap()
    sel2_d = nc.dram_tensor("sel2", [C * S2, S2], f32, kind="ExternalInput").ap()
    out_d = nc.dram_tensor("out", [S, T], f32, kind="ExternalOutput").ap()

    with tile.TileContext(nc) as tc:
        with (
            tc.tile_pool(name="w", bufs=8) as wp,
            tc.tile_pool(name="w2", bufs=1) as wp2,
            tc.tile_pool(name="cst", bufs=1) as cp,
            tc.tile_pool(name="junk", bufs=3) as jp,
            tc.tile_pool(name="acc", bufs=3) as accp,
            tc.tile_pool(name="fin", bufs=1) as fp,
            tc.tile_pool(name="ps", bufs=1, space="PSUM") as pp,
        ):
            possb = cp.tile([128, TF], f32, tag="possb")
            possb2 = cp.tile([C * S2, 1], f32, tag="possb2")
            sel = cp.tile([128, S], f32, tag="sel")
            sel2 = cp.tile([C * S2, S2], f32, tag="sel2")
            # partials: lateral col + afferent col per tile
            plat = cp.tile([128, TF], f32, tag="plat")
            paff = cp.tile([128, TF], f32, tag="paff")
            p2 = cp.tile([C * S2, 2], f32, tag="p2")
            nc.gpsimd.dma_start(possb[:], possb_d[:])
            nc.gpsimd.dma_start(possb2[:], possb2_d[:])
            nc.gpsimd.dma_start(sel[:], sel_d[:])
            nc.gpsimd.dma_start(sel2[:], sel2_d[:])

            def lateral_act(w, scale_ap, out_col):
                # one ScalarE op over the merged We|-Wi region
                j = jp.tile([128, 2 * FW], f32, tag="jlat")
                nc.scalar.activation(
                    j[:w.shape[0], :], w[:, 0:2 * FW], AF.Copy,
                    scale=scale_ap, accum_out=out_col,
                )

            def lateral_dve(w, scale_ap, out_col):
                r = accp.tile([128, 1], f32, tag="r")
                nc.vector.tensor_reduce(
                    r[:w.shape[0], :], w[:, 0:2 * FW], axis=AX.X, op=AL.add
                )
                nc.vector.tensor_mul(out_col, r[:w.shape[0], :], scale_ap)

            def afferent(w, out_col):
                prod = jp.tile([128, FA], f32, tag="prod")
                nc.vector.tensor_mul(
                    prod[:w.shape[0], :], w[:, 2 * FW:2 * FW + FA],
                    w[:, 2 * FW + FA:COLS],
                )
                nc.vector.tensor_reduce(
                    out_col, prod[:w.shape[0], :], axis=AX.X, op=AL.add
                )

            for t in range(TF):
                w = wp.tile([128, COLS], f32, tag="w")
                nc.sync.dma_start(w[:], big[t])
                if t in DVE_TILES:
                    lateral_dve(w, possb[:, t:t + 1], plat[:, t:t + 1])
                else:
                    lateral_act(w, possb[:, t:t + 1], plat[:, t:t + 1])
                afferent(w, paff[:, t:t + 1])

            # last tile: 2 units on 32 partitions (p = c*S2 + s)
            w2 = wp2.tile([C * S2, COLS], f32, tag="w2")
            nc.sync.dma_start(w2[:], big2_d[:])
            lateral_act(w2, possb2[:, 0:1], p2[:, 0:1])
            afferent(w2, p2[:, 1:2])

            # Channel sum via 0/1-selector matmuls on PE; lateral and
            # afferent partials accumulate into the same PSUM region.
            psum = pp.tile([S, TF], f32, tag="ps")
            psum2 = pp.tile([S2, 1], f32, tag="ps2")
            nc.tensor.matmul(psum[:], sel[:], plat[:], start=True, stop=False)
            nc.tensor.matmul(psum[:], sel[:], paff[:], start=False, stop=True)
            nc.tensor.matmul(psum2[:], sel2[:], p2[:, 0:1],
                             start=True, stop=False)
            nc.tensor.matmul(psum2[:], sel2[:], p2[:, 1:2],
                             start=False, stop=True)

            res = fp.tile([S, T], f32, tag="res")
            nc.vector.memset(res[:], 0.0)
            nc.vector.tensor_scalar_max(res[:, 0:TF], psum[:], 0.0)
            nc.vector.tensor_scalar_max(res[0:S2, TF:T], psum2[:], 0.0)
            nc.sync.dma_start(out_d[:], res[:])

    nc.compile()
    return nc


def _get_program():
    if "nc" not in _PROGRAM_CACHE:
        _PROGRAM_CACHE["nc"] = _build_program()
    return _PROGRAM_CACHE["nc"]


def _prep_in_maps(inputs):
    x = np.asarray(inputs["x"], dtype=np.float32)
    prev = np.asarray(inputs["prev_activity"], dtype=np.float32)
    wa = np.asarray(inputs["afferent_weights"], dtype=np.float32).reshape(C, UNITS, FA)
    we = np.asarray(inputs["ex_lateral_weights"], dtype=np.float32).reshape(C, UNITS, FW)
    wi = np.asarray(inputs["in_lateral_weights"], dtype=np.float32).reshape(C, UNITS, FW)
    rx = np.asarray(inputs["rx"]).astype(np.int64)
    ry = np.asarray(inputs["ry"]).astype(np.int64)

    u = np.arange(RF)
    ix = rx[:, None] + u                     # [GX, RF]
    iy = ry[:, None] + u                     # [GY, RF]
    px = x[:, ix, :]                         # [C, GX, RF, IMG]
    patches = px[:, :, :, iy]                # [C, GX, RF, GY, RF]
    patches = np.ascontiguousarray(patches.transpose(0, 1, 3, 2, 4))
    patches = patches.reshape(C, UNITS, FA)
    prevf = prev.reshape(C, UNITS)

    sel = (np.arange(128)[:, None] % S == np.arange(S)[None, :]).astype(np.float32)
    sel2 = (np.arange(C * S2)[:, None] % S2 == np.arange(S2)[None, :]).astype(np.float32)
    blk = np.concatenate([we, -wi, wa, patches], axis=2)   # [C, UNITS, COLS]

    in_maps = []
    for k in range(N_CORES):
        n0 = k * PER_CORE
        s = blk[:, n0:n0 + TF * S]                          # [C, 160, COLS]
        big = s.reshape(C, TF, S, COLS).transpose(1, 0, 2, 3).reshape(TF, C * S, COLS)
        big2 = blk[:, n0 + TF * S:n0 + PER_CORE].reshape(C * S2, COLS)
        pv = prevf[:, n0:n0 + TF * S]
        pv = pv.reshape(C, TF, S).transpose(0, 2, 1).reshape(C * S, TF)
        pv2 = prevf[:, n0 + TF * S:n0 + PER_CORE].reshape(C * S2, 1)
        in_maps.append({
            "big": np.ascontiguousarray(big),
            "big2": np.ascontiguousarray(big2),
            "possb": np.ascontiguousarray(GAMMA * pv),
            "possb2": np.ascontiguousarray(GAMMA * pv2),
            "sel": sel,
            "sel2": sel2,
        })
    return in_maps


def _assemble_output(results):
    act = np.empty(UNITS, np.float32)
    for k in range(N_CORES):
        o = np.asarray(results[k]["out"])            # [S, T]
        loc = o[:, 0:TF].T.reshape(TF * S)           # unit n_local = 8t + s
        act[k * PER_CORE:k * PER_CORE + TF * S] = loc
        act[k * PER_CORE + TF * S:(k + 1) * PER_CORE] = o[0:S2, TF]
    out = np.broadcast_to(act.reshape(1, GX, GY), (C, GX, GY))
    return np.ascontiguousarray(out, dtype=np.float32)


def kernel(**inputs):
    nc = _get_program()
    in_maps = _prep_in_maps(inputs)
    res = run_bass_kernel_spmd(nc, in_maps, core_ids=list(range(N_CORES)))
    return _assemble_output(res.results)


# revision 7
# speedup vs baseline: 1.0360x; 1.0121x over previous
"""Trainium2 Bass kernel for nn_CortexNetwork (dense_cnn, memory-bound).

Reference computation:
    patches[c,i,j,u,v] = x[c, rx[i]+u, ry[j]+v]
    aff[i,j] = sum_{c,u,v} patches * Wa
    exc[i,j] = sum_c prev[c,i,j] * sum_{x,y} We[c,i,j,x,y]   (inh likewise, Wi)
    out      = broadcast_c(relu(aff + 0.9*exc - 0.9*inh))

Strategy: tensor-parallel over the 36x36=1296 grid units, 162 units per
core on 8 cores; every reduction is unit-local so there are no
collectives.  The host lays each core's data out as 20 tiles of
[128 partitions = 16 channels x 8 units,
 3744 columns  = We(1296) | -Wi(1296) | Wa(576) | patch(576)]
plus one 32-partition tile for the 2 leftover units, so the device sees
one linear ~1.9MB DMA per tile.  Wi is negated on the host so the whole
lateral term is one reduction: 0.9*prev * sum(We|-Wi).  The free-dim
reductions are split across ScalarE (activation with scale=0.9*prev and
accum_out) and VectorE (tensor_reduce + per-partition multiply), with
ownership interleaved over tiles so both engines drain with the DMA
stream; all afferent products run on VectorE.  The final sum over the
16 channel partitions is a 0/1-selector matmul on the tensor engine,
then relu.
"""

import numpy as np

import concourse.bass as bass
import concourse.bacc as bacc
import concourse.mybir as mybir
from concourse import tile
from concourse.bass_utils import run_bass_kernel_spmd

N_CORES = 8
C = 16
GX = GY = 36
RF = 24
IMG = 64
GAMMA = 0.9

UNITS = GX * GY                  # 1296
PER_CORE = UNITS // N_CORES      # 162
S = 8                            # units per full tile (partition dim C*S=128)
TF = PER_CORE // S               # 20 full tiles
S2 = PER_CORE - TF * S           # 2 units in the last (32-partition) tile
T = TF + 1                       # 21 tiles total
FW = GX * GY                     # lateral free size per channel: 1296
FA = RF * RF                     # afferent free size per channel: 576
COLS = 2 * FW + 2 * FA           # 3744
# Full tiles whose lateral reduction runs on VectorE, spread through the
# stream so ScalarE and VectorE drain together; the rest go to ScalarE.
DVE_TILES = (2, 6, 9, 13, 16, 19)

_PROGRAM_CACHE = {}


def _build_program():
    f32 = mybir.dt.float32
    AL = mybir.AluOpType
    AF = mybir.ActivationFunctionType
    AX = mybir.AxisListType

    nc = bacc.Bacc(
        "TRN2", target_bir_lowering=False, debug=False, num_devices=N_CORES
    )
    big = nc.dram_tensor("big", [TF, 128, COLS], f32, kind="ExternalInput").ap()
    big2_d = nc.dram_tensor("big2", [C * S2, COLS], f32, kind="ExternalInput").ap()
    possb_d = nc.dram_tensor("possb", [128, TF], f32, kind="ExternalInput").ap()
    possb2_d = nc.dram_tensor("possb2", [C * S2, 1], f32, kind="ExternalInput").ap()
    sel_d = nc.dram_tensor("sel", [128, S], f32, kind="ExternalInput").ap()
    sel2_d = nc.dram_tensor("sel2", [C * S2, S2], f32, kind="ExternalInput").ap()
    out_d = nc.dram_tensor("out", [S, T], f32, kind="ExternalOutput").ap()

    with tile.TileContext(nc) as tc:
        with (
            tc.tile_pool(name="w", bufs=8) as wp,
            tc.tile_pool(name="w2", bufs=1) as wp2,
            tc.tile_pool(name="cst", bufs=1) as cp,
            tc.tile_pool(name="junk", bufs=3) as jp,
            tc.tile_pool(name="acc", bufs=3) as accp,
            tc.tile_pool(name="fin", bufs=1) as fp,
            tc.tile_pool(name="ps", bufs=1, space="PSUM") as pp,
        ):
            possb = cp.tile([128, TF], f32, tag="possb")
            possb2 = cp.tile([C * S2, 1], f32, tag="possb2")
            sel = cp.tile([128, S], f32, tag="sel")
            sel2 = cp.tile([C * S2, S2], f32, tag="sel2")
            # partials: lateral col + afferent col per tile
            plat = cp.tile([128, TF], f32, tag="plat")
            paff = cp.tile([128, TF], f32, tag="paff")
            p2 = cp.tile([C * S2, 2], f32, tag="p2")
            nc.gpsimd.dma_start(possb[:], possb_d[:])
            nc.gpsimd.dma_start(possb2[:], possb2_d[:])
            nc.gpsimd.dma_start(sel[:], sel_d[:])
            nc.gpsimd.dma_start(sel2[:], sel2_d[:])

            def lateral_act(w, scale_ap, out_col):
                # one ScalarE op over the merged We|-Wi region
                j = jp.tile([128, 2 * FW], f32, tag="jlat")
                nc.scalar.activation(
                    j[:w.shape[0], :], w[:, 0:2 * FW], AF.Copy,
                    scale=scale_ap, accum_out=out_col,
                )

            def lateral_dve(w, scale_ap, out_col):
                r = accp.tile([128, 1], f32, tag="r")
                nc.vector.tensor_reduce(
                    r[:w.shape[0], :], w[:, 0:2 * FW], axis=AX.X, op=AL.add
                )
                nc.vector.tensor_mul(out_col, r[:w.shape[0], :], scale_ap)

            def afferent(w, out_col):
                prod = jp.tile([128, FA], f32, tag="prod")
                nc.vector.tensor_mul(
                    prod[:w.shape[0], :], w[:, 2 * FW:2 * FW + FA],
                    w[:, 2 * FW + FA:COLS],
                )
                nc.vector.tensor_reduce(
                    out_col, prod[:w.shape[0], :], axis=AX.X, op=AL.add
                )

            # The 32-partition leftover tile transfers slowly (few DMA
            # engines cover 32 partitions), so issue it first on the
            # SWDGE ring where it hides under the main HWDGE stream.
            w2 = wp2.tile([C * S2, COLS], f32, tag="w2")
            nc.gpsimd.dma_start(w2[:], big2_d[:])
            lateral_act(w2, possb2[:, 0:1], p2[:, 0:1])
            afferent(w2, p2[:, 1:2])

            for t in range(TF):
                w = wp.tile([128, COLS], f32, tag="w")
                nc.sync.dma_start(w[:], big[t])
                if t in DVE_TILES:
                    lateral_dve(w, possb[:, t:t + 1], plat[:, t:t + 1])
                else:
                    lateral_act(w, possb[:, t:t + 1], plat[:, t:t + 1])
                afferent(w, paff[:, t:t + 1])

            # Channel sum via 0/1-selector matmuls on PE; lateral and
            # afferent partials accumulate into the same PSUM region.
            psum = pp.tile([S, TF], f32, tag="ps")
            psum2 = pp.tile([S2, 1], f32, tag="ps2")
            nc.tensor.matmul(psum[:], sel[:], plat[:], start=True, stop=False)
            nc.tensor.matmul(psum[:], sel[:], paff[:], start=False, stop=True)
            nc.tensor.matmul(psum2[:], sel2[:], p2[:, 0:1],
                             start=True, stop=False)
            nc.tensor.matmul(psum2[:], sel2[:], p2[:, 1:2],
                             start=False, stop=True)

            res = fp.tile([S, T], f32, tag="res")
            nc.vector.memset(res[:], 0.0)
            nc.vector.tensor_scalar_max(res[:, 0:TF], psum[:], 0.0)
            nc.vector.tensor_scalar_max(res[0:S2, TF:T], psum2[:], 0.0)
            nc.sync.dma_start(out_d[:], res[:])

    nc.compile()
    return nc


def _get_program():
    if "nc" not in _PROGRAM_CACHE:
        _PROGRAM_CACHE["nc"] = _build_program()
    return _PROGRAM_CACHE["nc"]


def _prep_in_maps(inputs):
    x = np.asarray(inputs["x"], dtype=np.float32)
    prev = np.asarray(inputs["prev_activity"], dtype=np.float32)
    wa = np.asarray(inputs["afferent_weights"], dtype=np.float32).reshape(C, UNITS, FA)
    we = np.asarray(inputs["ex_lateral_weights"], dtype=np.float32).reshape(C, UNITS, FW)
    wi = np.asarray(inputs["in_lateral_weights"], dtype=np.float32).reshape(C, UNITS, FW)
    rx = np.asarray(inputs["rx"]).astype(np.int64)
    ry = np.asarray(inputs["ry"]).astype(np.int64)

    u = np.arange(RF)
    ix = rx[:, None] + u                     # [GX, RF]
    iy = ry[:, None] + u                     # [GY, RF]
    px = x[:, ix, :]                         # [C, GX, RF, IMG]
    patches = px[:, :, :, iy]                # [C, GX, RF, GY, RF]
    patches = np.ascontiguousarray(patches.transpose(0, 1, 3, 2, 4))
    patches = patches.reshape(C, UNITS, FA)
    prevf = prev.reshape(C, UNITS)

    sel = (np.arange(128)[:, None] % S == np.arange(S)[None, :]).astype(np.float32)
    sel2 = (np.arange(C * S2)[:, None] % S2 == np.arange(S2)[None, :]).astype(np.float32)
    blk = np.concatenate([we, -wi, wa, patches], axis=2)   # [C, UNITS, COLS]

    in_maps = []
    for k in range(N_CORES):
        n0 = k * PER_CORE
        s = blk[:, n0:n0 + TF * S]                          # [C, 160, COLS]
        big = s.reshape(C, TF, S, COLS).transpose(1, 0, 2, 3).reshape(TF, C * S, COLS)
        big2 = blk[:, n0 + TF * S:n0 + PER_CORE].reshape(C * S2, COLS)
        pv = prevf[:, n0:n0 + TF * S]
        pv = pv.reshape(C, TF, S).transpose(0, 2, 1).reshape(C * S, TF)
        pv2 = prevf[:, n0 + TF * S:n0 + PER_CORE].reshape(C * S2, 1)
        in_maps.append({
            "big": np.ascontiguousarray(big),
            "big2": np.ascontiguousarray(big2),
            "possb": np.ascontiguousarray(GAMMA * pv),
            "possb2": np.ascontiguousarray(GAMMA * pv2),
            "sel": sel,
            "sel2": sel2,
        })
    return in_maps


def _assemble_output(results):
    act = np.empty(UNITS, np.float32)
    for k in range(N_CORES):
        o = np.asarray(results[k]["out"])            # [S, T]
        loc = o[:, 0:TF].T.reshape(TF * S)           # unit n_local = 8t + s
        act[k * PER_CORE:k * PER_CORE + TF * S] = loc
        act[k * PER_CORE + TF * S:(k + 1) * PER_CORE] = o[0:S2, TF]
    out = np.broadcast_to(act.reshape(1, GX, GY), (C, GX, GY))
    return np.ascontiguousarray(out, dtype=np.float32)


def kernel(**inputs):
    nc = _get_program()
    in_maps = _prep_in_maps(inputs)
    res = run_bass_kernel_spmd(nc, in_maps, core_ids=list(range(N_CORES)))
    return _assemble_output(res.results)
